# revision 14
# baseline (speedup 1.0000x reference)
"""Trainium2 Bass kernel for nn_CoreAttention (S=2048, B=1, H=16, D=128).

Sharding: 16 heads across 8 NeuronCores (2 heads/core, tensor parallel).

Per head (everything stays feature-major; host supplies bf16 Q^T/NF,
K^T, V^T per head):
    qmt    = Wqk^T (Q^T/NF)              (bf16 PE, fp32 PSUM, DVE cast)
    kmt    = Wqk^T K^T                   (bf16 PE)
    vaug_j = [V_j Wv | ones]             (bf16, [s,e] layout + ones col)
    per key-block i:
      scoresT[k,q] = kmt_i^T @ qmt       (bf16 PE; causal: only q >= i*128)
      expT_i = exp(scoresT)              (ACT, PSUM->SBUF bf16)
      diag block of expT_i *= 0/1 mask   (GpSimd, zeroes the causal upper)
    per query-block i (interleaved two steps behind scores):
      ctx_aug[q,0:129] = sum_j expT_j(q-block i)^T @ vaug_j   (PE)
        -> cols 0:128 = unnormalized context, col 128 = softmax denom
      ctx = ctx_aug[:,0:128] * (1/ctx_aug[:,128])  (DVE recip + scalar mul)
      DMA ctx -> out, batched 4 query blocks per transfer

The stationary operand of the PV matmul is the exp'd score block, so the
ones column of vaug yields the softmax denominator for free and the
output lands in [q, e] layout -- no separate row-sum pass, no PE
transposes.  PV matmuls are interleaved with the scores matmuls at
instruction granularity so their per-matmul LDWEIGHTS (the PV cadence
limiter) hides under the 512-column score streams.

exp() runs without max-subtraction: scores fit comfortably in bf16/fp32
(max observed exp(score) ~1e4), matching the reference's masked softmax
to rounding error.
"""

import sys
from contextlib import ExitStack

import numpy as np

for _p in ("/opt/trn_rl_repo",):
    if _p not in sys.path:
        sys.path.insert(0, _p)

import ml_dtypes
import concourse.bass as bass
import concourse.tile as tile
from concourse import bacc, mybir
from concourse.bass_utils import run_bass_kernel_spmd

S, B, H, D = 2048, 1, 16, 128
HPC = 2  # heads per core
NCORES = 8
NB = S // 128  # 16 seq blocks of 128
NF = float(np.sqrt(2048.0 / 16.0))  # NORM_FACTOR
VSTRIDE = 132  # per-key-block stride in vaug: 128 v cols + 1 ones + 3 pad
NCONST = 5  # wqk h0, wqk h1, wv h0, wv h1, mask01

F32 = mybir.dt.float32
BF16 = mybir.dt.bfloat16
AF = mybir.ActivationFunctionType


def build_program() -> bass.Bass:
    nc = bacc.Bacc(
        "TRN2", target_bir_lowering=False, debug=False, num_devices=NCORES
    )

    qt_d = nc.dram_tensor("qt", [HPC, D, S], BF16, kind="ExternalInput")
    kt_d = nc.dram_tensor("kt", [HPC, D, S], BF16, kind="ExternalInput")
    vt_d = nc.dram_tensor("vt", [HPC, D, S], BF16, kind="ExternalInput")
    consts_d = nc.dram_tensor("consts", [NCONST, D, D], BF16, kind="ExternalInput")
    out_d = nc.dram_tensor("out", [HPC, S, D], F32, kind="ExternalOutput")

    with tile.TileContext(nc) as tc, ExitStack() as ctx:
        cpool = ctx.enter_context(tc.tile_pool(name="const", bufs=1))
        sb = ctx.enter_context(tc.tile_pool(name="sb", bufs=1))
        ps = ctx.enter_context(tc.tile_pool(name="ps", bufs=1, space="PSUM"))

        consts = cpool.tile([D, NCONST * D], BF16)
        nc.scalar.dma_start(
            consts[:].rearrange("p (n c) -> p n c", n=NCONST),
            consts_d[:].rearrange("n p c -> p n c"),
        )
        gb = {h: consts[:, h * D : (h + 1) * D] for h in range(HPC)}
        wvb = {h: consts[:, (2 + h) * D : (3 + h) * D] for h in range(HPC)}
        mask01 = consts[:, 4 * D : 5 * D]

        qtb, ktb, vtb, kmt, vaug, expt, osb = {}, {}, {}, {}, {}, {}, {}

        # ---- input DMA: 1024-col chunks, need-first order ----------------
        for h in range(HPC):
            qtb[h] = sb.tile([D, S], BF16, tag="qtb", bufs=2, name=f"qtb{h}")
            ktb[h] = sb.tile([D, S], BF16, tag="ktb", bufs=2, name=f"ktb{h}")
        # q on the sync queue, k on the gpsimd queue: the per-queue DMA
        # rate is the startup bottleneck, so stream them in parallel, with
        # small leading chunks so the first scores matmul fires early.
        for sl in (slice(0, 512), slice(512, 1024), slice(1024, 2048)):
            for h in range(HPC):
                nc.sync.dma_start(qtb[h][:, sl], qt_d[h][:, sl])
                nc.gpsimd.dma_start(ktb[h][:, sl], kt_d[h][:, sl])
        for h in range(HPC):
            vtb[h] = sb.tile([D, S], BF16, tag="vtb", bufs=2, name=f"vtb{h}")
            (nc.sync if h == 0 else nc.gpsimd).dma_start(vtb[h][:], vt_d[h])

        # vaug ones backdrop (cols j*VSTRIDE+128.. stay 1.0 after v copies)
        for h in range(HPC):
            vaug[h] = sb.tile(
                [D, NB * VSTRIDE], BF16, tag="vaug", bufs=2, name=f"vaug{h}"
            )
            nc.gpsimd.memset(vaug[h][:], 1.0)

        # ---- k projection through G = Wqk Wqk^T / NF (q stays raw) -------
        # scores = Q G K^T with G symmetric, so only K needs projecting and
        # the scores matmul streams raw Q straight from its input DMA.
        for h in range(HPC):
            kmt[h] = sb.tile([D, S], BF16, tag="kmt", bufs=2, name=f"kmt{h}")

        def kproj(h, lo, cw):
            p = ps.tile(
                [D, 512], F32, tag="pv", bufs=4, name=f"kproj_{h}_{lo}"
            )
            for c2 in range(0, cw, 512):
                nc.tensor.matmul(
                    p[:, c2 % 512 : c2 % 512 + 512] if cw > 512 else p[:],
                    gb[h],
                    ktb[h][:, lo + c2 : lo + c2 + 512],
                    skip_group_check=True,
                )
                nc.vector.tensor_copy(
                    kmt[h][:, lo + c2 : lo + c2 + 512],
                    p[:] if cw > 512 else p[:],
                )

        # ---- thunk builders ---------------------------------------------
        def scores_thunks(h, i):
            """PE thunks for key block i of head h; ACT exp and the gpsimd
            diag-mask multiply are bundled after the last MM of each chunk."""
            w = S - i * 128
            expt[(h, i)] = sb.tile(
                [D, w], BF16, tag=f"expt{i}", bufs=2, name=f"expt_h{h}_{i}"
            )
            thunks = []
            if i == 0:
                bounds = [(0, 512), (512, 512), (1024, 1024)]
            else:
                bounds = [
                    (c * 1024, min(1024, w - c * 1024))
                    for c in range((w + 1023) // 1024)
                ]
            for c, (lo, cw) in enumerate(bounds):
                mms = [(c2, min(c2 + 512, cw)) for c2 in range(0, cw, 512)]

                def chunk_thunk(h=h, i=i, c=c, lo=lo, cw=cw, mms=mms):
                    scp = ps.tile(
                        [D, cw], F32, tag="mm1024", bufs=2, name=f"sc_{h}_{i}_{c}"
                    )
                    for c2, ce in mms:
                        nc.tensor.matmul(
                            scp[:, c2:ce],
                            kmt[h][:, i * 128 : (i + 1) * 128],
                            qtb[h][:, i * 128 + lo + c2 : i * 128 + lo + ce],
                            skip_group_check=True,
                        )
                    nc.scalar.activation(
                        expt[(h, i)][:, lo : lo + cw], scp[:], AF.Exp
                    )
                    if c == 0:
                        dg = expt[(h, i)][:, 0:128]
                        nc.gpsimd.tensor_mul(dg, dg, mask01)

                thunks.append(chunk_thunk)
            return thunks

        def pv_thunks(h, i):
            """PE thunks for the PV accumulation of query block i of head h;
            normalize + batched output DMA bundled after the last pair."""
            pvp = ps.tile([D, 512], F32, tag="pv", bufs=4, name=f"pv_{h}_{i}")
            thunks = []
            for j in range(i + 1):

                def pair_thunk(h=h, i=i, j=j, pvp=pvp):
                    nc.tensor.matmul(
                        pvp[:, 0:129],
                        expt[(h, j)][:, (i - j) * 128 : (i - j) * 128 + 128],
                        vaug[h][:, j * VSTRIDE : j * VSTRIDE + 129],
                        start=(j == 0),
                        stop=(j == i),
                        skip_group_check=True,
                    )
                    if j == i:
                        r = sb.tile([D, 1], F32, tag="rec", bufs=4, name=f"rec_{h}_{i}")
                        nc.vector.reciprocal(r[:], pvp[:, 128:129])
                        if i % 4 == 0:
                            osb[h] = sb.tile(
                                [D, 512], F32, tag="osb", bufs=4, name=f"osb_{h}_{i}"
                            )
                        nc.vector.tensor_scalar_mul(
                            osb[h][:, (i % 4) * 128 : (i % 4 + 1) * 128],
                            pvp[:, 0:128],
                            r[:],
                        )
                        # groups 0-2: one DMA per 4 blocks; last group: per
                        # 2 blocks so the kernel tail isn't one big transfer
                        if i < 12 and i % 4 == 3:
                            g = i // 4
                            nc.sync.dma_start(
                                out_d[h, g * 512 : (g + 1) * 512, :].rearrange(
                                    "(b s) e -> s b e", b=4
                                ),
                                osb[h][:].rearrange("p (b e) -> p b e", b=4),
                            )
                        elif i >= 12 and i % 2 == 1:
                            q0 = (i - 1) * 128
                            o0 = ((i - 1) % 4) * 128
                            nc.sync.dma_start(
                                out_d[h, q0 : q0 + 256, :].rearrange(
                                    "(b s) e -> s b e", b=2
                                ),
                                osb[h][:, o0 : o0 + 256].rearrange(
                                    "p (b e) -> p b e", b=2
                                ),
                            )

                thunks.append(pair_thunk)
            return thunks

        def vproj_thunks(h):
            """PE thunks for the v projection (LDW-heavy N=128 pairs);
            batched strided casts into vaug bundled after each 8-block run."""
            thunks = []
            for c in range(2):

                def head_thunk(h=h, c=c, first=True):
                    pass

                vp_holder = {}

                def mk(h=h, c=c, j8=0, vp_holder=vp_holder):
                    def t():
                        if j8 == 0:
                            vp_holder["t"] = ps.tile(
                                [D, 1024], F32, tag="mm1024", bufs=2,
                                name=f"vp_{h}_{c}",
                            )
                        vp = vp_holder["t"]
                        j = c * 8 + j8
                        nc.tensor.matmul(
                            vp[:, j8 * 128 : (j8 + 1) * 128],
                            vtb[h][:, j * 128 : (j + 1) * 128],
                            wvb[h],
                            skip_group_check=True,
                        )
                        if j8 == 7:
                            dst = (
                                vaug[h][:, c * 8 * VSTRIDE : (c + 1) * 8 * VSTRIDE]
                                .rearrange("p (j x) -> p j x", x=VSTRIDE)[:, :, 0:128]
                            )
                            src = vp[:].rearrange("p (j x) -> p j x", x=128)
                            nc.vector.tensor_copy(dst, src)

                    return t

                for j8 in range(8):
                    thunks.append(mk(h=h, c=c, j8=j8))
            return thunks

        def interleave(primary, secondary):
            """Emit primary (score) thunks spread evenly through the
            secondary (PV) thunk stream."""
            if not primary:
                for t in secondary:
                    t()
                return
            step = max(1, (len(secondary) + len(primary) - 1) // len(primary))
            si = 0
            for pt in primary:
                pt()
                for _ in range(step):
                    if si < len(secondary):
                        secondary[si]()
                        si += 1
            while si < len(secondary):
                secondary[si]()
                si += 1

        # ---- main interleaved loop --------------------------------------
        # k projection in DMA-arrival order, interleaved with the first
        # scores chunks so exp starts as soon as data lands.
        sc0h0 = scores_thunks(0, 0)  # chunks [512, 512, 1024]
        sc0h1 = scores_thunks(1, 0)
        kproj(0, 0, 512)
        kproj(1, 0, 512)
        sc0h0[0]()
        sc0h1[0]()
        kproj(0, 512, 512)
        kproj(1, 512, 512)
        sc0h0[1]()
        sc0h1[1]()
        for lo in (1024, 1536):
            kproj(0, lo, 512)
            kproj(1, lo, 512)
        sc0h0[2]()
        sc0h1[2]()
        # PV trails scores by 2 blocks early (ACT latency slack), then by 1
        # late so the un-overlapped tail after the last scores is short.
        pv_next = 0
        for i in range(1, NB):
            sc = scores_thunks(0, i) + scores_thunks(1, i)
            other = []
            if i == 1:
                other = vproj_thunks(0) + vproj_thunks(1)
            delay = 2 if i < 10 else 1
            while pv_next <= i - delay:
                other += pv_thunks(0, pv_next) + pv_thunks(1, pv_next)
                pv_next += 1
            interleave(sc, other)
        while pv_next < NB:
            for t in pv_thunks(0, pv_next) + pv_thunks(1, pv_next):
                t()
            pv_next += 1

    nc.compile()
    return nc


_NC_CACHE = None


def _get_program():
    global _NC_CACHE
    if _NC_CACHE is None:
        _NC_CACHE = build_program()
    return _NC_CACHE


def make_in_maps(query_layer, key_layer, value_layer, svd_qk, svd_v):
    qt = query_layer[:, 0].transpose(1, 2, 0).astype(ml_dtypes.bfloat16)
    kt = key_layer[:, 0].transpose(1, 2, 0).astype(ml_dtypes.bfloat16)
    vt = value_layer[:, 0].transpose(1, 2, 0).astype(ml_dtypes.bfloat16)
    wqk = np.asarray(svd_qk, dtype=np.float32)
    g = (wqk @ wqk.transpose(0, 2, 1) / NF).astype(ml_dtypes.bfloat16)
    wv = np.asarray(svd_v, dtype=np.float32).astype(ml_dtypes.bfloat16)

    r = np.arange(D)
    mask01 = (r[:, None] <= r[None, :]).astype(ml_dtypes.bfloat16)

    in_maps = []
    for c in range(NCORES):
        hs = slice(c * HPC, c * HPC + HPC)
        consts = np.stack(
            [g[c * HPC], g[c * HPC + 1], wv[c * HPC], wv[c * HPC + 1], mask01]
        )
        in_maps.append(
            {
                "qt": np.ascontiguousarray(qt[hs]),
                "kt": np.ascontiguousarray(kt[hs]),
                "vt": np.ascontiguousarray(vt[hs]),
                "consts": consts,
            }
        )
    return in_maps


def assemble_output(results):
    out = np.empty((S, B, H * D), dtype=np.float32)
    for c in range(NCORES):
        o = results[c]["out"]  # [HPC, S, D]
        for hl in range(HPC):
            h = c * HPC + hl
            out[:, 0, h * D : (h + 1) * D] = o[hl]
    return out


def kernel(query_layer, key_layer, value_layer, attention_mask, svd_qk, svd_v):
    nc = _get_program()
    in_maps = make_in_maps(query_layer, key_layer, value_layer, svd_qk, svd_v)
    res = run_bass_kernel_spmd(nc, in_maps, list(range(NCORES))).results
    return assemble_output(res)


# revision 15
# speedup vs baseline: 1.0573x; 1.0573x over previous
"""Trainium2 Bass kernel for nn_CoreAttention (S=2048, B=1, H=16, D=128).

Sharding: 16 heads across 8 NeuronCores (2 heads/core, tensor parallel).

Per head (everything stays feature-major; host supplies bf16 Q^T/NF,
K^T, V^T per head):
    qmt    = Wqk^T (Q^T/NF)              (bf16 PE, fp32 PSUM, DVE cast)
    kmt    = Wqk^T K^T                   (bf16 PE)
    vaug_j = [V_j Wv | ones]             (bf16, [s,e] layout + ones col)
    per key-block i:
      scoresT[k,q] = kmt_i^T @ qmt       (bf16 PE; causal: only q >= i*128)
      expT_i = exp(scoresT)              (ACT, PSUM->SBUF bf16)
      diag block of expT_i *= 0/1 mask   (GpSimd, zeroes the causal upper)
    per query-block i (interleaved two steps behind scores):
      ctx_aug[q,0:129] = sum_j expT_j(q-block i)^T @ vaug_j   (PE)
        -> cols 0:128 = unnormalized context, col 128 = softmax denom
      ctx = ctx_aug[:,0:128] * (1/ctx_aug[:,128])  (DVE recip + scalar mul)
      DMA ctx -> out, batched 4 query blocks per transfer

The stationary operand of the PV matmul is the exp'd score block, so the
ones column of vaug yields the softmax denominator for free and the
output lands in [q, e] layout -- no separate row-sum pass, no PE
transposes.  PV matmuls are interleaved with the scores matmuls at
instruction granularity so their per-matmul LDWEIGHTS (the PV cadence
limiter) hides under the 512-column score streams.

exp() runs without max-subtraction: scores fit comfortably in bf16/fp32
(max observed exp(score) ~1e4), matching the reference's masked softmax
to rounding error.
"""

import sys
from contextlib import ExitStack

import numpy as np

for _p in ("/opt/trn_rl_repo",):
    if _p not in sys.path:
        sys.path.insert(0, _p)

import ml_dtypes
import concourse.bass as bass
import concourse.tile as tile
from concourse import bacc, mybir
from concourse.bass_utils import run_bass_kernel_spmd

S, B, H, D = 2048, 1, 16, 128
HPC = 2  # heads per core
NCORES = 8
NB = S // 128  # 16 seq blocks of 128
NF = float(np.sqrt(2048.0 / 16.0))  # NORM_FACTOR
VSTRIDE = 132  # per-key-block stride in vaug: 128 v cols + 1 ones + 3 pad
NCONST = 5  # wqk h0, wqk h1, wv h0, wv h1, mask01

F32 = mybir.dt.float32
BF16 = mybir.dt.bfloat16
AF = mybir.ActivationFunctionType


def build_program() -> bass.Bass:
    nc = bacc.Bacc(
        "TRN2", target_bir_lowering=False, debug=False, num_devices=NCORES
    )

    qt_d = nc.dram_tensor("qt", [HPC, D, S], BF16, kind="ExternalInput")
    kt_d = nc.dram_tensor("kt", [HPC, D, S], BF16, kind="ExternalInput")
    vt_d = nc.dram_tensor("vt", [HPC, D, S], BF16, kind="ExternalInput")
    consts_d = nc.dram_tensor("consts", [NCONST, D, D], BF16, kind="ExternalInput")
    out_d = nc.dram_tensor("out", [HPC, S, D], F32, kind="ExternalOutput")

    with tile.TileContext(nc) as tc, ExitStack() as ctx:
        cpool = ctx.enter_context(tc.tile_pool(name="const", bufs=1))
        sb = ctx.enter_context(tc.tile_pool(name="sb", bufs=1))
        ps = ctx.enter_context(tc.tile_pool(name="ps", bufs=1, space="PSUM"))

        consts = cpool.tile([D, NCONST * D], BF16)
        nc.scalar.dma_start(
            consts[:].rearrange("p (n c) -> p n c", n=NCONST),
            consts_d[:].rearrange("n p c -> p n c"),
        )
        gb = {h: consts[:, h * D : (h + 1) * D] for h in range(HPC)}
        wvb = {h: consts[:, (2 + h) * D : (3 + h) * D] for h in range(HPC)}
        mask01 = consts[:, 4 * D : 5 * D]

        qtb, ktb, vtb, kmt, vaug, expt, osb = {}, {}, {}, {}, {}, {}, {}

        # ---- input DMA: 1024-col chunks, need-first order ----------------
        for h in range(HPC):
            qtb[h] = sb.tile([D, S], BF16, tag="qtb", bufs=2, name=f"qtb{h}")
            ktb[h] = sb.tile([D, S], BF16, tag="ktb", bufs=2, name=f"ktb{h}")
        # q on the sync queue, k on the gpsimd queue: the per-queue DMA
        # rate is the startup bottleneck, so stream them in parallel, with
        # small leading chunks so the first scores matmul fires early.
        # kmt[1024:] is not consumed until key block 8, so those k chunks
        # yield their queue slot to v.
        for h in range(HPC):
            vtb[h] = sb.tile([D, S], BF16, tag="vtb", bufs=2, name=f"vtb{h}")
        for sl in (slice(0, 512), slice(512, 1024)):
            for h in range(HPC):
                nc.sync.dma_start(qtb[h][:, sl], qt_d[h][:, sl])
                nc.gpsimd.dma_start(ktb[h][:, sl], kt_d[h][:, sl])
        for h in range(HPC):
            nc.sync.dma_start(qtb[h][:, 1024:2048], qt_d[h][:, 1024:2048])
        for sl in (slice(0, 1024), slice(1024, 2048)):
            nc.sync.dma_start(vtb[0][:, sl], vt_d[0][:, sl])
            nc.gpsimd.dma_start(vtb[1][:, sl], vt_d[1][:, sl])
        for h in range(HPC):
            nc.gpsimd.dma_start(ktb[h][:, 1024:2048], kt_d[h][:, 1024:2048])

        # vaug ones backdrop (cols j*VSTRIDE+128.. stay 1.0 after v copies)
        for h in range(HPC):
            vaug[h] = sb.tile(
                [D, NB * VSTRIDE], BF16, tag="vaug", bufs=2, name=f"vaug{h}"
            )
            nc.gpsimd.memset(vaug[h][:], 1.0)

        # ---- k projection through G = Wqk Wqk^T / NF (q stays raw) -------
        # scores = Q G K^T with G symmetric, so only K needs projecting and
        # the scores matmul streams raw Q straight from its input DMA.
        for h in range(HPC):
            kmt[h] = sb.tile([D, S], BF16, tag="kmt", bufs=2, name=f"kmt{h}")

        def kproj(h, lo, cw):
            p = ps.tile(
                [D, 512], F32, tag="pv", bufs=4, name=f"kproj_{h}_{lo}"
            )
            for c2 in range(0, cw, 512):
                nc.tensor.matmul(
                    p[:, c2 % 512 : c2 % 512 + 512] if cw > 512 else p[:],
                    gb[h],
                    ktb[h][:, lo + c2 : lo + c2 + 512],
                    skip_group_check=True,
                )
                nc.vector.tensor_copy(
                    kmt[h][:, lo + c2 : lo + c2 + 512],
                    p[:] if cw > 512 else p[:],
                )

        # ---- thunk builders ---------------------------------------------
        def scores_thunks(h, i):
            """PE thunks for key block i of head h; ACT exp and the gpsimd
            diag-mask multiply are bundled after the last MM of each chunk."""
            w = S - i * 128
            expt[(h, i)] = sb.tile(
                [D, w], BF16, tag=f"expt{i}", bufs=2, name=f"expt_h{h}_{i}"
            )
            thunks = []
            if i == 0:
                bounds = [(0, 512), (512, 512), (1024, 1024)]
            else:
                bounds = [
                    (c * 1024, min(1024, w - c * 1024))
                    for c in range((w + 1023) // 1024)
                ]
            for c, (lo, cw) in enumerate(bounds):
                mms = [(c2, min(c2 + 512, cw)) for c2 in range(0, cw, 512)]

                def chunk_thunk(h=h, i=i, c=c, lo=lo, cw=cw, mms=mms):
                    scp = ps.tile(
                        [D, cw], F32, tag="mm1024", bufs=2, name=f"sc_{h}_{i}_{c}"
                    )
                    for c2, ce in mms:
                        nc.tensor.matmul(
                            scp[:, c2:ce],
                            kmt[h][:, i * 128 : (i + 1) * 128],
                            qtb[h][:, i * 128 + lo + c2 : i * 128 + lo + ce],
                            skip_group_check=True,
                        )
                    nc.scalar.activation(
                        expt[(h, i)][:, lo : lo + cw], scp[:], AF.Exp
                    )
                    if c == 0:
                        dg = expt[(h, i)][:, 0:128]
                        nc.gpsimd.tensor_mul(dg, dg, mask01)

                thunks.append(chunk_thunk)
            return thunks

        def pv_thunks(h, i):
            """PE thunks for the PV accumulation of query block i of head h;
            normalize + batched output DMA bundled after the last pair."""
            pvp = ps.tile([D, 512], F32, tag="pv", bufs=4, name=f"pv_{h}_{i}")
            thunks = []
            for j in range(i + 1):

                def pair_thunk(h=h, i=i, j=j, pvp=pvp):
                    nc.tensor.matmul(
                        pvp[:, 0:129],
                        expt[(h, j)][:, (i - j) * 128 : (i - j) * 128 + 128],
                        vaug[h][:, j * VSTRIDE : j * VSTRIDE + 129],
                        start=(j == 0),
                        stop=(j == i),
                        skip_group_check=True,
                    )
                    if j == i:
                        r = sb.tile([D, 1], F32, tag="rec", bufs=4, name=f"rec_{h}_{i}")
                        nc.vector.reciprocal(r[:], pvp[:, 128:129])
                        if i % 4 == 0:
                            osb[h] = sb.tile(
                                [D, 512], F32, tag="osb", bufs=4, name=f"osb_{h}_{i}"
                            )
                        nc.vector.tensor_scalar_mul(
                            osb[h][:, (i % 4) * 128 : (i % 4 + 1) * 128],
                            pvp[:, 0:128],
                            r[:],
                        )
                        # groups 0-2: one DMA per 4 blocks; last group: per
                        # 2 blocks so the kernel tail isn't one big transfer
                        if i < 12 and i % 4 == 3:
                            g = i // 4
                            nc.sync.dma_start(
                                out_d[h, g * 512 : (g + 1) * 512, :].rearrange(
                                    "(b s) e -> s b e", b=4
                                ),
                                osb[h][:].rearrange("p (b e) -> p b e", b=4),
                            )
                        elif i >= 12 and i % 2 == 1:
                            q0 = (i - 1) * 128
                            o0 = ((i - 1) % 4) * 128
                            nc.sync.dma_start(
                                out_d[h, q0 : q0 + 256, :].rearrange(
                                    "(b s) e -> s b e", b=2
                                ),
                                osb[h][:, o0 : o0 + 256].rearrange(
                                    "p (b e) -> p b e", b=2
                                ),
                            )

                thunks.append(pair_thunk)
            return thunks

        def vproj_thunks(h):
            """PE thunks for the v projection (LDW-heavy N=128 pairs);
            batched strided casts into vaug bundled after each 8-block run."""
            thunks = []
            for c in range(2):

                def head_thunk(h=h, c=c, first=True):
                    pass

                vp_holder = {}

                def mk(h=h, c=c, j8=0, vp_holder=vp_holder):
                    def t():
                        if j8 == 0:
                            vp_holder["t"] = ps.tile(
                                [D, 1024], F32, tag="mm1024", bufs=2,
                                name=f"vp_{h}_{c}",
                            )
                        vp = vp_holder["t"]
                        j = c * 8 + j8
                        nc.tensor.matmul(
                            vp[:, j8 * 128 : (j8 + 1) * 128],
                            vtb[h][:, j * 128 : (j + 1) * 128],
                            wvb[h],
                            skip_group_check=True,
                        )
                        if j8 == 7:
                            dst = (
                                vaug[h][:, c * 8 * VSTRIDE : (c + 1) * 8 * VSTRIDE]
                                .rearrange("p (j x) -> p j x", x=VSTRIDE)[:, :, 0:128]
                            )
                            src = vp[:].rearrange("p (j x) -> p j x", x=128)
                            nc.vector.tensor_copy(dst, src)

                    return t

                for j8 in range(8):
                    thunks.append(mk(h=h, c=c, j8=j8))
            return thunks

        def interleave(primary, secondary):
            """Emit primary (score) thunks spread evenly through the
            secondary (PV) thunk stream."""
            if not primary:
                for t in secondary:
                    t()
                return
            step = max(1, (len(secondary) + len(primary) - 1) // len(primary))
            si = 0
            for pt in primary:
                pt()
                for _ in range(step):
                    if si < len(secondary):
                        secondary[si]()
                        si += 1
            while si < len(secondary):
                secondary[si]()
                si += 1

        # ---- main interleaved loop --------------------------------------
        # k projection in DMA-arrival order, interleaved with the first
        # scores chunks so exp starts as soon as data lands.
        sc0h0 = scores_thunks(0, 0)  # chunks [512, 512, 1024]
        sc0h1 = scores_thunks(1, 0)
        kproj(0, 0, 512)
        kproj(1, 0, 512)
        sc0h0[0]()
        sc0h1[0]()
        kproj(0, 512, 512)
        kproj(1, 512, 512)
        sc0h0[1]()
        sc0h1[1]()
        sc0h0[2]()
        sc0h1[2]()
        # PV trails scores by 2 blocks early (ACT latency slack), then by 1
        # late so the un-overlapped tail after the last scores is short.
        pv_next = 0
        for i in range(1, NB):
            sc = scores_thunks(0, i) + scores_thunks(1, i)
            other = []
            if i == 2:
                other += vproj_thunks(0) + vproj_thunks(1)
            if i == 3:

                def late_kproj():
                    for lo in (1024, 1536):
                        kproj(0, lo, 512)
                        kproj(1, lo, 512)

                other += [late_kproj]
            delay = 2 if i < 10 else 1
            while pv_next <= i - delay:
                other += pv_thunks(0, pv_next) + pv_thunks(1, pv_next)
                pv_next += 1
            if i <= 2:
                for t in sc:
                    t()
                for t in other:
                    t()
            else:
                interleave(sc, other)
        while pv_next < NB:
            for t in pv_thunks(0, pv_next) + pv_thunks(1, pv_next):
                t()
            pv_next += 1

    nc.compile()
    return nc


_NC_CACHE = None


def _get_program():
    global _NC_CACHE
    if _NC_CACHE is None:
        _NC_CACHE = build_program()
    return _NC_CACHE


def make_in_maps(query_layer, key_layer, value_layer, svd_qk, svd_v):
    qt = query_layer[:, 0].transpose(1, 2, 0).astype(ml_dtypes.bfloat16)
    kt = key_layer[:, 0].transpose(1, 2, 0).astype(ml_dtypes.bfloat16)
    vt = value_layer[:, 0].transpose(1, 2, 0).astype(ml_dtypes.bfloat16)
    wqk = np.asarray(svd_qk, dtype=np.float32)
    g = (wqk @ wqk.transpose(0, 2, 1) / NF).astype(ml_dtypes.bfloat16)
    wv = np.asarray(svd_v, dtype=np.float32).astype(ml_dtypes.bfloat16)

    r = np.arange(D)
    mask01 = (r[:, None] <= r[None, :]).astype(ml_dtypes.bfloat16)

    in_maps = []
    for c in range(NCORES):
        hs = slice(c * HPC, c * HPC + HPC)
        consts = np.stack(
            [g[c * HPC], g[c * HPC + 1], wv[c * HPC], wv[c * HPC + 1], mask01]
        )
        in_maps.append(
            {
                "qt": np.ascontiguousarray(qt[hs]),
                "kt": np.ascontiguousarray(kt[hs]),
                "vt": np.ascontiguousarray(vt[hs]),
                "consts": consts,
            }
        )
    return in_maps


def assemble_output(results):
    out = np.empty((S, B, H * D), dtype=np.float32)
    for c in range(NCORES):
        o = results[c]["out"]  # [HPC, S, D]
        for hl in range(HPC):
            h = c * HPC + hl
            out[:, 0, h * D : (h + 1) * D] = o[hl]
    return out


def kernel(query_layer, key_layer, value_layer, attention_mask, svd_qk, svd_v):
    nc = _get_program()
    in_maps = make_in_maps(query_layer, key_layer, value_layer, svd_qk, svd_v)
    res = run_bass_kernel_spmd(nc, in_maps, list(range(NCORES))).results
    return assemble_output(res)
